# revision 1
# baseline (speedup 1.0000x reference)
"""AdaFocal Trainium2 kernel, class-sorted layout.

The loss is a sum over rows, so kernel() may reorder rows freely. Host
pre-sorts rows by target class into a per-core-identical slice schedule:
slice s (128 rows across partitions) of EVERY core holds rows of class
schedule[s], so the per-row gather input[i, t_i] becomes a compile-time
strided column copy x[:, ja:jb, t*] -- one DVE op per run of equal-class
slices. Rows that don't fill a whole uniform slice land in a mixed tail
handled by the general per-slice (iota==t)*x gather.

Per chunk [128 part x k rows x 128 cls]:
  ScalarE: e = exp(x) -> bf16   (no max-subtract: |x| small, f32-safe)
  Pool:    fold1+fold2 of e (tensor_tensor add, bf16)
  DVE:     s = tensor_reduce(fold2), xt = column copies (+ tail stt)
Epilogue: logpt = xt - ln(s), pt, binning with baked consts, loss sum,
AllReduce across the 8 cores.
"""

import sys

for _p in ("/opt/trn_rl_repo", "/opt/pypackages"):
    if _p not in sys.path:
        sys.path.insert(0, _p)

import numpy as np

from concourse import bass, mybir
from concourse.bass_utils import run_bass_kernel_spmd

N_CORES = 8
P = 128
C = 128
EPS = 1e-20

ALU = mybir.AluOpType
ACT = mybir.ActivationFunctionType
F32 = mybir.dt.float32
BF16 = mybir.dt.bfloat16


def build_graph(rows_per_core: int, k: int, bin_uppers_vals, gammas_vals,
                schedule=None):
    """schedule: list of class ids, one per leading uniform slice (slice s
    holds 128 rows all of class schedule[s]); remaining slices are mixed
    (gathered via stt with runtime targets). None = all mixed."""
    assert rows_per_core % (P * k) == 0
    chunks = rows_per_core // (P * k)
    n_slices = chunks * k
    if schedule is None:
        schedule = []
    assert len(schedule) <= n_slices
    uppers = [float(v) for v in bin_uppers_vals]
    gammas = [float(v) for v in gammas_vals]
    uniform = all(g == gammas[0] for g in gammas)
    need_pow = (not uniform) or abs(gammas[0]) != 1.0

    nc = bass.Bass(num_devices=N_CORES)

    x_ext = nc.declare_dram_parameter("input", [rows_per_core, C], F32, isOutput=False)
    t_ext = nc.declare_dram_parameter("targf", [rows_per_core], F32, isOutput=False)
    iota_ext = nc.declare_dram_parameter("iota", [P, C], F32, isOutput=False)
    out_ext = nc.declare_dram_parameter("out", [P, 1], F32, isOutput=True)

    x_view = x_ext[:].rearrange("(c p j) w -> c p j w", p=P, j=k)
    t_view_pre = t_ext[:].rearrange("(c p j) -> p c j", p=P, j=k)

    cols = chunks * k
    NBUF = 4

    x_buf = [nc.alloc_sbuf_tensor(f"x_buf{b}", [P, k, C], F32) for b in range(NBUF)]
    e_buf = [nc.alloc_sbuf_tensor(f"e_buf{b}", [P, k, C], BF16) for b in range(NBUF)]
    f1_buf = [nc.alloc_sbuf_tensor(f"f1_buf{b}", [P, k, C // 2], BF16)
              for b in range(NBUF)]
    t_all = nc.alloc_sbuf_tensor("t_all", [P, cols], F32)
    iota_sb = nc.alloc_sbuf_tensor("iota_sb", [P, C], F32)
    s_all = nc.alloc_sbuf_tensor("s_all", [P, cols], F32)
    xt_all = nc.alloc_sbuf_tensor("xt_all", [P, cols], F32)
    lns = nc.alloc_sbuf_tensor("lns", [P, cols], F32)
    logpt = nc.alloc_sbuf_tensor("logpt", [P, cols], F32)
    ptb = nc.alloc_sbuf_tensor("ptb", [P, cols], F32)
    ab = nc.alloc_sbuf_tensor("ab", [P, cols], F32)
    sc1 = nc.alloc_sbuf_tensor("sc1", [P, cols], F32)
    sc2 = nc.alloc_sbuf_tensor("sc2", [P, cols], F32)
    mgb = None if uniform else nc.alloc_sbuf_tensor("mgb", [P, cols], F32)
    loss_part = nc.alloc_sbuf_tensor("loss_part", [P, 1], F32)

    iota_sem = nc.alloc_semaphore("iota_sem")
    tpre_sem = nc.alloc_semaphore("tpre_sem")
    x_sem = [nc.alloc_semaphore(f"x_sem{b}") for b in range(NBUF)]
    xts = [nc.alloc_semaphore(f"xts{b}") for b in range(NBUF)]
    act_done = nc.alloc_semaphore("act_done")
    dve_x = nc.alloc_semaphore("dve_x")
    dve_s = nc.alloc_semaphore("dve_s")
    pool_done = nc.alloc_semaphore("pool_done")
    pfence = nc.alloc_semaphore("pfence")
    ep_sem = nc.alloc_semaphore("ep_sem")
    cc_sem = nc.alloc_semaphore("cc_sem")
    ccin_sem = nc.alloc_semaphore("ccin_sem")
    red_sem = nc.alloc_semaphore("red_sem")
    fin_sem = nc.alloc_semaphore("fin_sem")

    E_LOSS = 8 if need_pow else 4

    def slice_class(s):
        return schedule[s] if s < len(schedule) else None

    DMA_RUN_MIN = 10**9  # xt DMA disabled: 4B/descriptor made DMA engines the ceiling

    def chunk_runs(c):
        runs = []  # (ja, jb, cls) uniform runs; cls None => mixed slice
        j = 0
        while j < k:
            cls = slice_class(c * k + j)
            j2 = j + 1
            while j2 < k and slice_class(c * k + j2) == cls:
                j2 += 1
            runs.append((j, j2, cls))
            j = j2
        return runs

    xt_dma_runs = {}   # c -> runs copied via DMA (disabled)
    xt_dve_runs = {}   # c -> runs for DVE (cls None => stt)
    xt_act_runs = {}   # c -> big uniform runs for ScalarE (ACT Copy)
    flip = 0
    for c in range(chunks):
        dve_r, act_r = [], []
        for ja, jb, cls in chunk_runs(c):
            if cls is not None and jb - ja >= 4:
                # balance strided-copy elements across DVE and ScalarE
                if flip % 2 == 0:
                    act_r.append((ja, jb, cls))
                else:
                    dve_r.append((ja, jb, cls))
                flip += 1
            else:
                dve_r.append((ja, jb, cls))
        xt_dma_runs[c] = []
        xt_dve_runs[c] = dve_r
        xt_act_runs[c] = act_r
    total_xt_dma = {b: 0 for b in range(NBUF)}

    with nc.Block(name="adafocal") as block:

        @block.sync
        def _(sync: bass.BassEngine):
            sync.dma_start(out=x_buf[0][:], in_=x_view[0]).then_inc(x_sem[0], 16)
            sync.dma_start(out=iota_sb[:], in_=iota_ext[:]).then_inc(iota_sem, 16)
            sync.dma_start(
                out=t_all[:].rearrange("p (c j) -> p c j", j=k), in_=t_view_pre
            ).then_inc(tpre_sem, 16)
            issued_xt = {b: 0 for b in range(NBUF)}

            def issue_xt(c):
                b2 = c % NBUF
                for ja, jb, cls in xt_dma_runs[c]:
                    with nc.allow_non_contiguous_dma(
                        reason="column gather: 4B/partition per slice"
                    ):
                        sync.dma_start(
                            out=xt_all[:, c * k + ja : c * k + jb],
                            in_=x_buf[b2][:, ja:jb, cls : cls + 1],
                        ).then_inc(xts[b2], 16)
                    issued_xt[b2] += 1

            for c in range(1, chunks):
                b = c % NBUF
                if c >= NBUF:
                    sync.wait_ge(act_done, c - NBUF + 1)
                    sync.wait_ge(dve_x, c - NBUF + 1)
                    if issued_xt[b]:
                        sync.wait_ge(xts[b], 16 * issued_xt[b])  # xt reads done
                sync.dma_start(out=x_buf[b][:], in_=x_view[c]).then_inc(x_sem[b], 16)
                if c >= 1 and xt_dma_runs[c - 1]:
                    sync.wait_ge(x_sem[(c - 1) % NBUF], 16 * ((c - 1) // NBUF + 1))
                    issue_xt(c - 1)
            if xt_dma_runs[chunks - 1]:
                sync.wait_ge(
                    x_sem[(chunks - 1) % NBUF], 16 * ((chunks - 1) // NBUF + 1)
                )
                issue_xt(chunks - 1)

        @block.scalar
        def _(scalar: bass.BassEngine):
            for c in range(chunks):
                b = c % NBUF
                scalar.wait_ge(x_sem[b], 16 * (c // NBUF + 1))
                if c >= NBUF:
                    if (c - NBUF) % 2 == 0:
                        scalar.wait_ge(pool_done, (c - NBUF) // 2 + 1)
                    else:
                        scalar.wait_ge(dve_s, c - NBUF + 1)
                inst = scalar.activation(
                    out=e_buf[b][:], in_=x_buf[b][:], func=ACT.Exp
                )
                for ja, jb, cls in xt_act_runs[c]:
                    inst = scalar.activation(
                        out=xt_all[:, c * k + ja : c * k + jb],
                        in_=x_buf[b][:, ja:jb, cls],
                        func=ACT.Copy,
                    )
                inst.then_inc(act_done, 1)

        @block.gpsimd
        def _(gpsimd: bass.BassEngine):
            nfold = 0
            for c in range(0, chunks, 2):  # even chunks only
                b = c % NBUF
                gpsimd.wait_ge(act_done, c + 1)
                if c >= NBUF:
                    gpsimd.wait_ge(dve_s, c - NBUF + 1)  # f1 free again
                nfold += 1
                gpsimd.tensor_tensor(
                    out=f1_buf[b][:],
                    in0=e_buf[b][:, :, 0 : C // 2],
                    in1=e_buf[b][:, :, C // 2 : C],
                    op=ALU.add,
                ).then_inc(pfence, 1)
                gpsimd.wait_ge(pfence, nfold)
                gpsimd.tensor_tensor(
                    out=f1_buf[b][:, :, 0 : C // 4],
                    in0=f1_buf[b][:, :, 0 : C // 4],
                    in1=f1_buf[b][:, :, C // 4 : C // 2],
                    op=ALU.add,
                ).then_inc(pool_done, 1)

        @block.vector
        def _(vector: bass.BassEngine):
            vector.wait_ge(iota_sem, 16)
            vector.wait_ge(tpre_sem, 16)
            for c in range(chunks):
                b = c % NBUF
                vector.wait_ge(act_done, c + 1)  # x (and e) landed
                last = None
                for ja, jb, cls in xt_dve_runs[c]:
                    if cls is not None:
                        last = vector.tensor_copy(
                            out=xt_all[:, c * k + ja : c * k + jb],
                            in_=x_buf[b][:, ja:jb, cls],
                        )
                    else:
                        for j in range(ja, jb):
                            s = c * k + j
                            last = vector.scalar_tensor_tensor(
                                out=x_buf[b][:, j, :],
                                in0=iota_sb[:],
                                scalar=t_all[:, s : s + 1],
                                in1=x_buf[b][:, j, :],
                                op0=ALU.is_equal,
                                op1=ALU.mult,
                                accum_out=xt_all[:, s : s + 1],
                            )
                if last is None:
                    last = vector.tensor_copy(
                        out=sc1[:, 0:1], in_=iota_sb[:, 0:1]
                    )  # keep dve_x cadence
                last.then_inc(dve_x, 1)
                if c % 2 == 0:
                    vector.wait_ge(pool_done, c // 2 + 1)
                    vector.tensor_reduce(
                        out=s_all[:, c * k : (c + 1) * k],
                        in_=f1_buf[b][:, :, 0 : C // 4],
                        axis=mybir.AxisListType.X,
                        op=ALU.add,
                    ).then_inc(dve_s, 1)
                else:
                    vector.tensor_reduce(
                        out=s_all[:, c * k : (c + 1) * k],
                        in_=e_buf[b][:],
                        axis=mybir.AxisListType.X,
                        op=ALU.add,
                    ).then_inc(dve_s, 1)

        # ---- epilogue ----

        @block.scalar
        def _(scalar: bass.BassEngine):
            scalar.wait_ge(dve_s, chunks)
            scalar.wait_ge(dve_x, chunks)
            scalar.wait_ge(act_done, chunks)  # own xt copies done (fence)
            for b in range(NBUF):
                if total_xt_dma[b]:
                    scalar.wait_ge(xts[b], 16 * total_xt_dma[b])
            scalar.activation(out=lns[:], in_=s_all[:], func=ACT.Ln).then_inc(
                ep_sem, 1
            )  # ep=1
            scalar.wait_ge(ep_sem, 2)
            scalar.activation(out=ptb[:], in_=logpt[:], func=ACT.Exp).then_inc(
                ep_sem, 1
            )  # ep=3
            if need_pow:
                scalar.wait_ge(ep_sem, 4)
                scalar.activation(out=sc2[:], in_=ab[:], func=ACT.Ln).then_inc(
                    ep_sem, 1
                )  # ep=5
                scalar.wait_ge(ep_sem, 6)
                scalar.activation(out=ab[:], in_=sc1[:], func=ACT.Exp).then_inc(
                    ep_sem, 1
                )  # ep=7

        @block.vector
        def _(vector: bass.BassEngine):
            vector.wait_ge(ep_sem, 1)
            vector.tensor_tensor(
                out=logpt[:], in0=xt_all[:], in1=lns[:], op=ALU.subtract
            ).then_inc(ep_sem, 1)  # ep=2
            vector.wait_ge(ep_sem, 3)
            if uniform:
                sgn = float(np.sign(gammas[0]))
                vector.tensor_scalar(
                    out=ab[:],
                    in0=ptb[:],
                    scalar1=-sgn,
                    scalar2=1.0,
                    op0=ALU.mult,
                    op1=ALU.add,
                )
                vector.drain()
                if need_pow:
                    mag = float(abs(gammas[0]))
                    vector.tensor_scalar(
                        out=ab[:], in0=ab[:], scalar1=1e-30, scalar2=None, op0=ALU.max
                    ).then_inc(ep_sem, 1)  # ep=4
                    vector.wait_ge(ep_sem, 5)
                    vector.tensor_scalar(
                        out=sc1[:], in0=sc2[:], scalar1=mag, scalar2=None, op0=ALU.mult
                    ).then_inc(ep_sem, 1)  # ep=6
                    vector.wait_ge(ep_sem, 7)
            else:
                vector.tensor_scalar(
                    out=sc2[:],
                    in0=ptb[:],
                    scalar1=0.0,
                    scalar2=gammas[0],
                    op0=ALU.mult,
                    op1=ALU.add,
                )
                for kk in range(len(uppers)):
                    dg = gammas[kk + 1] - gammas[kk]
                    if dg == 0.0:
                        continue
                    vector.drain()
                    vector.tensor_scalar(
                        out=sc1[:],
                        in0=ptb[:],
                        scalar1=uppers[kk],
                        scalar2=None,
                        op0=ALU.is_ge,
                    )
                    vector.drain()
                    vector.scalar_tensor_tensor(
                        out=sc2[:],
                        in0=sc1[:],
                        scalar=dg,
                        in1=sc2[:],
                        op0=ALU.mult,
                        op1=ALU.add,
                    )
                vector.drain()
                vector.tensor_scalar(
                    out=sc1[:], in0=sc2[:], scalar1=0.0, scalar2=None, op0=ALU.is_gt
                )
                vector.tensor_scalar(
                    out=ab[:], in0=sc2[:], scalar1=0.0, scalar2=None, op0=ALU.is_lt
                )
                vector.drain()
                vector.tensor_tensor(out=sc1[:], in0=sc1[:], in1=ab[:], op=ALU.subtract)
                vector.drain()
                vector.tensor_tensor(out=mgb[:], in0=sc2[:], in1=sc1[:], op=ALU.mult)
                vector.tensor_tensor(out=ab[:], in0=sc1[:], in1=ptb[:], op=ALU.mult)
                vector.drain()
                vector.tensor_scalar(
                    out=ab[:],
                    in0=ab[:],
                    scalar1=-1.0,
                    scalar2=1.0,
                    op0=ALU.mult,
                    op1=ALU.add,
                )
                vector.drain()
                vector.tensor_scalar(
                    out=ab[:], in0=ab[:], scalar1=EPS, scalar2=None, op0=ALU.add
                )
                vector.drain()
                vector.tensor_scalar(
                    out=ab[:], in0=ab[:], scalar1=1e-30, scalar2=None, op0=ALU.max
                ).then_inc(ep_sem, 1)  # ep=4
                vector.wait_ge(ep_sem, 5)
                vector.tensor_tensor(
                    out=sc1[:], in0=sc2[:], in1=mgb[:], op=ALU.mult
                ).then_inc(ep_sem, 1)  # ep=6
                vector.wait_ge(ep_sem, 7)
            vector.tensor_tensor(out=sc1[:], in0=ab[:], in1=logpt[:], op=ALU.mult)
            vector.drain()
            vector.tensor_reduce(
                out=loss_part[:], in_=sc1[:], axis=mybir.AxisListType.X, op=ALU.add
            ).then_inc(ep_sem, 1)  # ep = E_LOSS

        @block.sync
        def _(sync: bass.BassEngine):
            sync.wait_ge(ep_sem, E_LOSS)
            sync.dma_start(out=out_ext[:], in_=loss_part[:]).then_inc(fin_sem, 16)
            sync.wait_ge(fin_sem, 16)

    return nc


def make_schedule(target, rows_per_core, k):
    """Per-core-identical slice schedule + per-core row permutations.

    Mixed (non-uniform) slices are interleaved evenly across chunks so their
    costlier per-row gather hides inside the DMA-bound steady state instead
    of serializing at the end of the run.
    """
    target = np.asarray(target)
    n_slices = rows_per_core // P
    chunks = n_slices // k
    counts = np.bincount(target, minlength=C)
    n_t = counts // (P * N_CORES)
    uni = [t for t in range(C) for _ in range(int(n_t[t]))]
    n_mixed = n_slices - len(uni)
    # reserve the last ceil-share slice positions of each chunk for mixed
    reserved = set()
    base, extra = divmod(n_mixed, chunks)
    for c in range(chunks):
        m = base + (1 if c < extra else 0)
        for j in range(k - m, k):
            reserved.add(c * k + j)
    schedule = [None] * n_slices
    it = iter(uni)
    for s in range(n_slices):
        if s not in reserved:
            schedule[s] = next(it)

    by_class = [np.flatnonzero(target == t) for t in range(C)]
    tail_rows = rows_per_core - P * len(uni)

    def slice_slots(s):
        c, j = divmod(s, k)
        return c * P * k + np.arange(P) * k + j

    leftovers = np.concatenate(
        [by_class[t][int(P * N_CORES * n_t[t]):] for t in range(C)]
    )
    assert leftovers.size == tail_rows * N_CORES
    perms = []
    for i in range(N_CORES):
        perm = np.empty(rows_per_core, dtype=np.int64)
        cls_pos = {t: 0 for t in range(C)}
        tail = leftovers[i * tail_rows : (i + 1) * tail_rows]
        tpos = 0
        for s in range(n_slices):
            t = schedule[s]
            if t is None:
                perm[slice_slots(s)] = tail[tpos : tpos + P]
                tpos += P
            else:
                u = cls_pos[t]
                rows_t = by_class[t][
                    i * int(P * n_t[t]) + u * P : i * int(P * n_t[t]) + (u + 1) * P
                ]
                perm[slice_slots(s)] = rows_t
                cls_pos[t] = u + 1
        perms.append(perm)
    return schedule, perms


_IOTA = None


def _iota_arr():
    global _IOTA
    if _IOTA is None:
        _IOTA = np.broadcast_to(np.arange(C, dtype=np.float32)[None, :], (P, C)).copy()
    return _IOTA


def kernel(input, target, bin_uppers, gammas, _k=32, **run_kwargs):
    input = np.asarray(input, dtype=np.float32)
    target = np.asarray(target)
    bin_uppers = np.asarray(bin_uppers, dtype=np.float32)
    gammas = np.asarray(gammas, dtype=np.float32)

    n = input.shape[0]
    assert n % N_CORES == 0
    rows = n // N_CORES
    assert rows % (P * _k) == 0

    schedule, perms = make_schedule(target, rows, _k)
    nc = build_graph(rows, _k, bin_uppers.tolist(), gammas.tolist(), schedule)

    iota = _iota_arr()
    targf = target.astype(np.float32)
    in_maps = []
    for i in range(N_CORES):
        in_maps.append(
            {
                "input": input[perms[i]],
                "targf": targf[perms[i]],
                "iota": iota,
            }
        )
    res = run_bass_kernel_spmd(
        nc, in_maps, core_ids=list(range(N_CORES)), **run_kwargs
    )
    total = -sum(
        float(res.results[i]["out"].astype(np.float64).sum()) for i in range(N_CORES)
    )
    return np.float32(total)



# revision 4
# speedup vs baseline: 1.1170x; 1.1170x over previous
"""AdaFocal Trainium2 kernel, v2: DMA-roofline design.

Per-row work: s = sum_c exp(x[r,c]); logpt = x[r,t_r] - ln s; pt = e^logpt;
loss += -(1 - sign(g)*pt + eps)^|g| * logpt  (g from pt's bin; all g equal
1.0 for this problem so the pow path folds away).

The target-column gather x[r, t_r] is extracted on the host (cheaper than
the class-sort row permutation the previous version used) and DMA'd in as
a dense [P, cols] tensor, so the device never gathers. Device pipeline per
chunk [128 part x k x 128 cls]:

  DMA  : x chunk (f32), even chunks on qSP (sync), odd on qAct (scalar) --
         two HWDGE queues so per-DMA completion stalls overlap.
  ACT  : e = exp(x) -> bf16 (no max-subtract: |x| small, f32-safe)
  Pool : fold e left+right halves -> f1 [P,k,64] bf16
  DVE  : s = tensor_reduce(f1) -> s_all

Epilogue (logpt/pt/loss) runs in col quarters: three quarters are injected
mid-stream (hidden under DMA), only the last quarter trails the final
chunk. Tail chunks shrink (16,8,8 slices) so the last chunk's chain is
short. A dummy Ln up front makes the single act-table load fetch
natural_log_exp_and_others (serves exp+ln+copy; no mid-stream reload).
"""

import sys

for _p in ("/opt/trn_rl_repo", "/opt/pypackages"):
    if _p not in sys.path:
        sys.path.insert(0, _p)

import numpy as np

from concourse import bass, mybir
from concourse.bass_utils import run_bass_kernel_spmd

N_CORES = 8
P = 128
C = 128
EPS = 1e-20

ALU = mybir.AluOpType
ACT = mybir.ActivationFunctionType
F32 = mybir.dt.float32
BF16 = mybir.dt.bfloat16

KMAX = 32
NBUF = 5


def make_ks(n_slices):
    """Chunk sizes in 128-row slices; shrinking tail shortens the last
    chunk's exp->fold->reduce chain."""
    tail = [16, 8, 8]
    body = (n_slices - sum(tail)) // KMAX
    assert body * KMAX + sum(tail) == n_slices
    return [KMAX] * body + tail


def build_graph(rows_per_core: int, bin_uppers_vals, gammas_vals):
    n_slices = rows_per_core // P
    ks = make_ks(n_slices)
    chunks = len(ks)
    offs = np.concatenate([[0], np.cumsum(ks)]).astype(int)  # col offsets
    cols = int(offs[-1])
    uppers = [float(v) for v in bin_uppers_vals]
    gammas = [float(v) for v in gammas_vals]
    uniform = all(g == gammas[0] for g in gammas)
    need_pow = (not uniform) or abs(gammas[0]) != 1.0

    # quarter boundaries for the split epilogue: after these chunk counts
    # the corresponding col range is fully reduced.
    NQ = 4
    qb = []  # (chunk_count_needed, col_lo, col_hi)
    for q in range(NQ):
        lo, hi = cols * q // NQ, cols * (q + 1) // NQ
        nchunk = int(np.searchsorted(offs, hi))  # offs[nchunk] == hi
        assert offs[nchunk] == hi
        qb.append((nchunk, lo, hi))

    nc = bass.Bass(num_devices=N_CORES)

    x_ext = nc.declare_dram_parameter("input", [rows_per_core, C], F32, isOutput=False)
    xt_ext = nc.declare_dram_parameter("xt", [P, cols], F32, isOutput=False)
    out_ext = nc.declare_dram_parameter("out", [P, NQ], F32, isOutput=True)

    x_flat = x_ext[:]

    def x_chunk_view(c):
        r0 = int(offs[c]) * P
        r1 = int(offs[c + 1]) * P
        return x_flat[r0:r1, :].rearrange("(p j) w -> p j w", p=P, j=ks[c])

    x_buf = [nc.alloc_sbuf_tensor(f"x_buf{b}", [P, KMAX, C], F32) for b in range(NBUF)]
    e_buf = [nc.alloc_sbuf_tensor(f"e_buf{b}", [P, KMAX, C], BF16) for b in range(NBUF)]
    f1_buf = [
        nc.alloc_sbuf_tensor(f"f1_buf{b}", [P, KMAX, C // 2], BF16)
        for b in range(NBUF)
    ]
    xt_all = nc.alloc_sbuf_tensor("xt_all", [P, cols], F32)
    s_all = nc.alloc_sbuf_tensor("s_all", [P, cols], F32)
    lns = nc.alloc_sbuf_tensor("lns", [P, cols], F32)
    logpt = nc.alloc_sbuf_tensor("logpt", [P, cols], F32)
    ptb = nc.alloc_sbuf_tensor("ptb", [P, cols], F32)
    ab = nc.alloc_sbuf_tensor("ab", [P, cols], F32)
    sc1 = nc.alloc_sbuf_tensor("sc1", [P, cols], F32)
    sc2 = nc.alloc_sbuf_tensor("sc2", [P, cols], F32)
    mgb = None if uniform else nc.alloc_sbuf_tensor("mgb", [P, cols], F32)
    loss_part = nc.alloc_sbuf_tensor("loss_part", [P, NQ], F32)

    x_sem = [nc.alloc_semaphore(f"x_sem{b}") for b in range(NBUF)]
    xt_sem = nc.alloc_semaphore("xt_sem")
    act_done = nc.alloc_semaphore("act_done")
    pool_done = nc.alloc_semaphore("pool_done")
    dve_s = nc.alloc_semaphore("dve_s")
    ep_sem = nc.alloc_semaphore("ep_sem")
    fin_sem = nc.alloc_semaphore("fin_sem")

    # epilogue sem budget per quarter
    EP_PER_Q = 8 if need_pow else 4

    def ep_base(q):
        return q * EP_PER_Q

    def x_sem_target(c):
        return 16 * (c // NBUF + 1)

    def emit_ep_scalar(scalar, q, stage):
        """Scalar-engine epilogue piece for quarter q.
        stage 0: Ln of s; stage 1: exp(logpt); (need_pow) stage 2: ln(ab);
        stage 3: exp(sc1)."""
        nchunk, lo, hi = qb[q]
        base = ep_base(q)
        if stage == 0:
            scalar.wait_ge(dve_s, nchunk)
            if q == 0:
                scalar.wait_ge(xt_sem, 16)
            scalar.activation(
                out=lns[:, lo:hi], in_=s_all[:, lo:hi], func=ACT.Ln
            ).then_inc(ep_sem, 1)  # base+1
        elif stage == 1:
            scalar.wait_ge(ep_sem, base + 2)
            scalar.activation(
                out=ptb[:, lo:hi], in_=logpt[:, lo:hi], func=ACT.Exp
            ).then_inc(ep_sem, 1)  # base+3
        elif stage == 2:
            scalar.wait_ge(ep_sem, base + 4)
            scalar.activation(
                out=sc2[:, lo:hi], in_=ab[:, lo:hi], func=ACT.Ln
            ).then_inc(ep_sem, 1)  # base+5
        elif stage == 3:
            scalar.wait_ge(ep_sem, base + 6)
            scalar.activation(
                out=ab[:, lo:hi], in_=sc1[:, lo:hi], func=ACT.Exp
            ).then_inc(ep_sem, 1)  # base+7
        else:
            raise AssertionError(stage)

    def emit_ep_vector(vector, q, stage):
        """Vector-engine epilogue piece for quarter q.
        stage 0: logpt = xt - ln s; stage 1: binning+loss chain."""
        nchunk, lo, hi = qb[q]
        base = ep_base(q)
        if stage == 0:
            vector.wait_ge(ep_sem, base + 1)
            vector.tensor_tensor(
                out=logpt[:, lo:hi],
                in0=xt_all[:, lo:hi],
                in1=lns[:, lo:hi],
                op=ALU.subtract,
            ).then_inc(ep_sem, 1)  # base+2
            return
        assert stage == 1
        vector.wait_ge(ep_sem, base + 3)
        if uniform:
            sgn = float(np.sign(gammas[0]))
            vector.tensor_scalar(
                out=ab[:, lo:hi],
                in0=ptb[:, lo:hi],
                scalar1=-sgn,
                scalar2=1.0,
                op0=ALU.mult,
                op1=ALU.add,
            )
            vector.drain()
            if need_pow:
                mag = float(abs(gammas[0]))
                vector.tensor_scalar(
                    out=ab[:, lo:hi],
                    in0=ab[:, lo:hi],
                    scalar1=1e-30,
                    scalar2=None,
                    op0=ALU.max,
                ).then_inc(ep_sem, 1)  # base+4
                vector.wait_ge(ep_sem, base + 5)
                vector.tensor_scalar(
                    out=sc1[:, lo:hi],
                    in0=sc2[:, lo:hi],
                    scalar1=mag,
                    scalar2=None,
                    op0=ALU.mult,
                ).then_inc(ep_sem, 1)  # base+6
                vector.wait_ge(ep_sem, base + 7)
        else:
            vector.tensor_scalar(
                out=sc2[:, lo:hi],
                in0=ptb[:, lo:hi],
                scalar1=0.0,
                scalar2=gammas[0],
                op0=ALU.mult,
                op1=ALU.add,
            )
            for kk in range(len(uppers)):
                dg = gammas[kk + 1] - gammas[kk]
                if dg == 0.0:
                    continue
                vector.drain()
                vector.tensor_scalar(
                    out=sc1[:, lo:hi],
                    in0=ptb[:, lo:hi],
                    scalar1=uppers[kk],
                    scalar2=None,
                    op0=ALU.is_ge,
                )
                vector.drain()
                vector.scalar_tensor_tensor(
                    out=sc2[:, lo:hi],
                    in0=sc1[:, lo:hi],
                    scalar=dg,
                    in1=sc2[:, lo:hi],
                    op0=ALU.mult,
                    op1=ALU.add,
                )
            vector.drain()
            vector.tensor_scalar(
                out=sc1[:, lo:hi],
                in0=sc2[:, lo:hi],
                scalar1=0.0,
                scalar2=None,
                op0=ALU.is_gt,
            )
            vector.tensor_scalar(
                out=ab[:, lo:hi],
                in0=sc2[:, lo:hi],
                scalar1=0.0,
                scalar2=None,
                op0=ALU.is_lt,
            )
            vector.drain()
            vector.tensor_tensor(
                out=sc1[:, lo:hi], in0=sc1[:, lo:hi], in1=ab[:, lo:hi],
                op=ALU.subtract,
            )
            vector.drain()
            vector.tensor_tensor(
                out=mgb[:, lo:hi], in0=sc2[:, lo:hi], in1=sc1[:, lo:hi],
                op=ALU.mult,
            )
            vector.tensor_tensor(
                out=ab[:, lo:hi], in0=sc1[:, lo:hi], in1=ptb[:, lo:hi],
                op=ALU.mult,
            )
            vector.drain()
            vector.tensor_scalar(
                out=ab[:, lo:hi],
                in0=ab[:, lo:hi],
                scalar1=-1.0,
                scalar2=1.0,
                op0=ALU.mult,
                op1=ALU.add,
            )
            vector.drain()
            vector.tensor_scalar(
                out=ab[:, lo:hi], in0=ab[:, lo:hi], scalar1=EPS, scalar2=None,
                op0=ALU.add,
            )
            vector.drain()
            vector.tensor_scalar(
                out=ab[:, lo:hi], in0=ab[:, lo:hi], scalar1=1e-30, scalar2=None,
                op0=ALU.max,
            ).then_inc(ep_sem, 1)  # base+4
            vector.wait_ge(ep_sem, base + 5)
            vector.tensor_tensor(
                out=sc1[:, lo:hi], in0=sc2[:, lo:hi], in1=mgb[:, lo:hi],
                op=ALU.mult,
            ).then_inc(ep_sem, 1)  # base+6
            vector.wait_ge(ep_sem, base + 7)
        vector.tensor_tensor(
            out=sc1[:, lo:hi], in0=ab[:, lo:hi], in1=logpt[:, lo:hi],
            op=ALU.mult,
        )
        vector.drain()
        vector.tensor_reduce(
            out=loss_part[:, q : q + 1],
            in_=sc1[:, lo:hi],
            axis=mybir.AxisListType.X,
            op=ALU.add,
        ).then_inc(ep_sem, 1)  # base+4 (uniform) / base+8 (pow)

    # scalar-side epilogue stage list per quarter (in ping-pong order)
    sc_stages = [0, 1] + ([2, 3] if need_pow else [])
    ve_stages = [0, 1]
    # injection chunk for mid-stream quarters: two chunks after the data
    # is ready, so the dve_s wait is free.  quarter 3 runs post-loop.
    inject_at = {}
    for q in range(NQ - 1):
        inject_at.setdefault(qb[q][0] + 1, []).append(q)

    with nc.Block(name="adafocal") as block:

        @block.sync
        def _(sync: bass.BassEngine):
            for c in range(0, min(NBUF, chunks), 2):
                sync.dma_start(
                    out=x_buf[c][:, 0 : ks[c], :], in_=x_chunk_view(c)
                ).then_inc(x_sem[c], 16)
            sync.dma_start(out=xt_all[:], in_=xt_ext[:]).then_inc(xt_sem, 16)
            for c in range(NBUF + (NBUF % 2), chunks, 2):  # even c >= NBUF
                sync.wait_ge(act_done, c - NBUF + 1)
                sync.dma_start(
                    out=x_buf[c % NBUF][:, 0 : ks[c], :], in_=x_chunk_view(c)
                ).then_inc(x_sem[c % NBUF], 16)
            # final output
            sync.wait_ge(ep_sem, NQ * EP_PER_Q)
            sync.dma_start(out=out_ext[:], in_=loss_part[:]).then_inc(fin_sem, 16)
            sync.wait_ge(fin_sem, 16)

        @block.scalar
        def _(scalar: bass.BassEngine):
            for c in range(1, min(NBUF, chunks), 2):
                scalar.dma_start(
                    out=x_buf[c][:, 0 : ks[c], :], in_=x_chunk_view(c)
                ).then_inc(x_sem[c], 16)
            # dummy Ln: the single table load fetches
            # natural_log_exp_and_others, which serves exp+ln+copy.
            scalar.activation(out=lns[:, 0:1], in_=s_all[:, 0:1], func=ACT.Ln)
            sc_q = []  # queued (q, stage) epilogue pieces
            for c in range(chunks):
                b = c % NBUF
                d = c + NBUF - 1  # odd-queue lookahead: exp(d-NBUF)=exp(c-1) done
                if d >= NBUF and d < chunks and d % 2 == 1:
                    scalar.dma_start(
                        out=x_buf[d % NBUF][:, 0 : ks[d], :], in_=x_chunk_view(d)
                    ).then_inc(x_sem[d % NBUF], 16)
                scalar.wait_ge(x_sem[b], x_sem_target(c))
                if c >= NBUF:
                    scalar.wait_ge(pool_done, c - NBUF + 1)
                scalar.activation(
                    out=e_buf[b][:, 0 : ks[c], :],
                    in_=x_buf[b][:, 0 : ks[c], :],
                    func=ACT.Exp,
                ).then_inc(act_done, 1)
                for q in inject_at.get(c, []):
                    sc_q.extend((q, st) for st in sc_stages)
                if sc_q:
                    emit_ep_scalar(scalar, *sc_q.pop(0))
            for q_st in sc_q:
                emit_ep_scalar(scalar, *q_st)
            for st in sc_stages:
                emit_ep_scalar(scalar, NQ - 1, st)

        @block.gpsimd
        def _(gpsimd: bass.BassEngine):
            for c in range(chunks):
                b = c % NBUF
                gpsimd.wait_ge(act_done, c + 1)
                if c >= NBUF:
                    gpsimd.wait_ge(dve_s, c - NBUF + 1)
                gpsimd.tensor_tensor(
                    out=f1_buf[b][:, 0 : ks[c], :],
                    in0=e_buf[b][:, 0 : ks[c], 0 : C // 2],
                    in1=e_buf[b][:, 0 : ks[c], C // 2 : C],
                    op=ALU.add,
                ).then_inc(pool_done, 1)

        @block.vector
        def _(vector: bass.BassEngine):
            ve_q = []
            for c in range(chunks):
                b = c % NBUF
                vector.wait_ge(pool_done, c + 1)
                vector.tensor_reduce(
                    out=s_all[:, int(offs[c]) : int(offs[c + 1])],
                    in_=f1_buf[b][:, 0 : ks[c], :],
                    axis=mybir.AxisListType.X,
                    op=ALU.add,
                ).then_inc(dve_s, 1)
                for q in inject_at.get(c, []):
                    ve_q.extend((q, st) for st in ve_stages)
                if ve_q:
                    emit_ep_vector(vector, *ve_q.pop(0))
            for q_st in ve_q:
                emit_ep_vector(vector, *q_st)
            for st in ve_stages:
                emit_ep_vector(vector, NQ - 1, st)

    return nc


def kernel(input, target, bin_uppers, gammas, **run_kwargs):
    input = np.asarray(input, dtype=np.float32)
    target = np.asarray(target)
    bin_uppers = np.asarray(bin_uppers, dtype=np.float32)
    gammas = np.asarray(gammas, dtype=np.float32)

    n = input.shape[0]
    assert n % N_CORES == 0
    rows = n // N_CORES
    assert rows % P == 0

    nc = build_graph(rows, bin_uppers.tolist(), gammas.tolist())

    ks = make_ks(rows // P)
    offs = np.concatenate([[0], np.cumsum(ks)]).astype(int)
    cols = int(offs[-1])

    xt_full = np.take_along_axis(
        input, target.astype(np.int64)[:, None], axis=1
    )[:, 0].astype(np.float32)

    in_maps = []
    for i in range(N_CORES):
        xt_core = xt_full[i * rows : (i + 1) * rows]
        xt2d = np.empty((P, cols), np.float32)
        for c, kc in enumerate(ks):
            r0 = int(offs[c]) * P
            xt2d[:, int(offs[c]) : int(offs[c + 1])] = xt_core[
                r0 : r0 + P * kc
            ].reshape(P, kc)
        in_maps.append(
            {
                "input": input[i * rows : (i + 1) * rows],
                "xt": xt2d,
            }
        )
    res = run_bass_kernel_spmd(
        nc, in_maps, core_ids=list(range(N_CORES)), **run_kwargs
    )
    total = -sum(
        float(res.results[i]["out"].astype(np.float64).sum()) for i in range(N_CORES)
    )
    return np.float32(total)
